# revision 16
# baseline (speedup 1.0000x reference)
"""AttnBlock (GroupNorm + single-head self-attention + residual) on 8 TRN2 cores.

Problem: x [2, 512, 16, 16, 16]; GroupNorm(32 groups) -> 1x1x1 conv Q/K/V ->
attention over N=4096 tokens -> output projection -> residual.

Sharding: 8 cores = 2 batches x 4 query-slices of 1024 tokens. Every core
redundantly computes GroupNorm + K + V^T for its batch (cheap vs attention),
and computes Q / scores / PV only for its 1024-token query slice. The
query-slice offset is baked into the DATA, not the program: core (b, s)
receives x[b] cyclically rolled by -1024*s along the token axis, so the
single SPMD program always works on tokens [0, 1024) — attention is
permutation-equivariant so the rolled output is exactly the out-slice.

Dataflow per core (everything stays in the "transposed" score layout so NO
on-chip transposes are needed):
  hn = groupnorm(x)                [c, t] f32 -> f32r in place
  K  = wk @ hn + bk                [c, j]  (lhsT = host-transposed wkT)
  Q  = wq @ hn[:, :1024] + bq      [c, i]
  VT = hn^T @ wvT + bv             [j, c]  (lhsT = hn)
  S^T[j, i] = K^T Q                via matmul(lhsT=K, rhs=Q)
  E^T = exp(S^T / sqrt(C))         bf16
  l[i] = ones^T @ E^T              PSUM accumulation over j
  O[c, i] = VT^T @ E^T             PSUM accumulation over j
  O /= l ;  out = wp @ O + bp + x
Matmuls run in float32r (full PE rate, ~1.5e-4 rel err) except the PV/ones
path which is bf16 (attention-weight noise averages out over the diffuse
softmax support).
"""

import sys

sys.path.insert(0, "/opt/trn_rl_repo")

import numpy as np

import concourse.bass as bass
import concourse.tile as tile
from concourse import bacc, mybir
from concourse.bass_utils import run_bass_kernel_spmd

F32 = mybir.dt.float32
F32R = mybir.dt.float32r
BF16 = mybir.dt.bfloat16
AF = mybir.ActivationFunctionType
OP = mybir.AluOpType

B, C = 2, 512
N = 16 * 16 * 16          # 4096 tokens
G, GS = 32, 16            # groups, channels per group
P, KC = 128, C // 128     # partitions, channel chunks (4)
NCORES = 8
SLICES = NCORES // B      # 4 query slices per batch
ISL = N // SLICES         # 1024 query tokens per core
IC = ISL // 512           # 512-wide i-chunks (2)
JT = N // P               # 32 j-tiles
JN = N // 512             # 8 j-chunks of 512
EPS = 1e-6
SCALE = 1.0 / np.sqrt(C)


def _emit(nc, tc):
    xd = nc.declare_dram_parameter("x", [C, N], F32R, isOutput=False)
    wqd = nc.declare_dram_parameter("wqT", [C, C], F32R, isOutput=False)
    wkd = nc.declare_dram_parameter("wkT", [C, C], F32R, isOutput=False)
    wvd = nc.declare_dram_parameter("wvT", [C, C], F32R, isOutput=False)
    wpd = nc.declare_dram_parameter("wpT", [C, C], F32R, isOutput=False)
    bqd = nc.declare_dram_parameter("bq", [P, KC], F32, isOutput=False)
    bkd = nc.declare_dram_parameter("bk", [P, KC], F32, isOutput=False)
    bvd = nc.declare_dram_parameter("bv_row", [1, C], F32, isOutput=False)
    bpd = nc.declare_dram_parameter("bp", [P, KC], F32, isOutput=False)
    gwd = nc.declare_dram_parameter("gnw", [P, KC], F32, isOutput=False)
    gbd = nc.declare_dram_parameter("gnb", [P, KC], F32, isOutput=False)
    indd = nc.declare_dram_parameter("ind", [P, P // GS], F32R, isOutput=False)
    indTd = nc.declare_dram_parameter("indT", [P // GS, P], F32R, isOutput=False)
    onesd = nc.declare_dram_parameter("ones_col", [1, P], F32R, isOutput=False)
    od = nc.declare_dram_parameter("out", [C, ISL], F32, isOutput=True)

    xre = xd[:, :].rearrange("(kc p) t -> p kc t", p=P)

    main_pool = tc.tile_pool(name="main", bufs=1)
    w_pool = tc.tile_pool(name="wp", bufs=1)
    et_pool = tc.tile_pool(name="etp", bufs=2)
    with main_pool as main, w_pool as wpool, et_pool as etp:
        # ---------------- load x + params ----------------
        x_t = main.tile([P, KC, N], F32R, tag="bigA")
        xf = x_t.bitcast(F32)
        for kc in range(KC):
            nc.sync.dma_start(out=x_t[:, kc, :], in_=xre[:, kc, :])

        bq_t = main.tile([P, KC], F32, tag="bq")
        bk_t = main.tile([P, KC], F32, tag="bk")
        bp_t = main.tile([P, KC], F32, tag="bp")
        gw_t = main.tile([P, KC], F32, tag="gw")
        gb_t = main.tile([P, KC], F32, tag="gb")
        nc.sync.dma_start(out=bq_t, in_=bqd[:, :])
        nc.sync.dma_start(out=bk_t, in_=bkd[:, :])
        nc.sync.dma_start(out=bp_t, in_=bpd[:, :])
        nc.sync.dma_start(out=gw_t, in_=gwd[:, :])
        nc.sync.dma_start(out=gb_t, in_=gbd[:, :])
        bv_b = main.tile([P, C], BF16, tag="bvb")
        nc.gpsimd.dma_start(out=bv_b, in_=bvd[:, :].to_broadcast((P, C)))

        # ---------------- GroupNorm ----------------
        SG = N // 512  # bn_stats subgroups per chunk
        stm = main.tile([P, KC, SG, 6], F32, tag="bnst")
        mv = main.tile([P, KC, 2], F32, tag="mv")
        for kc in range(KC):
            for s in range(SG):
                nc.vector.bn_stats(
                    out=stm[:, kc, s, :], in_=xf[:, kc, s * 512 : (s + 1) * 512]
                )
            nc.vector.bn_aggr(out=mv[:, kc, :], in_=stm[:, kc, :, :])

        # statsm: [P, 2, KC] — block 0 = channel means, block 1 = E[x^2]
        statsm = main.tile([P, 2, KC], F32, tag="statsm")
        nc.vector.tensor_copy(out=statsm[:, 0, :], in_=mv[:, :, 0])
        nc.vector.tensor_tensor(statsm[:, 1, :], mv[:, :, 0], mv[:, :, 0], OP.mult)
        nc.vector.tensor_tensor(statsm[:, 1, :], statsm[:, 1, :], mv[:, :, 1], OP.add)
        statsm_r = main.tile([P, 2 * KC], F32R, tag="statsm_r")
        nc.vector.tensor_copy(out=statsm_r, in_=statsm)

        # group indicator matrices (8 groups per 128-channel chunk), host-provided
        GPC = P // GS  # 8
        ind = main.tile([P, GPC], F32R, tag="ind")
        nc.sync.dma_start(out=ind, in_=indd[:, :])
        indT = main.tile([GPC, P], F32R, tag="indT")
        nc.sync.dma_start(out=indT, in_=indTd[:, :])

        eps_t = main.tile([GPC, 1], F32, tag="eps")
        nc.vector.memset(eps_t, EPS)

        a_t = main.tile([P, KC], F32, tag="a_t")
        b2_t = main.tile([P, KC], F32, tag="b2_t")

        with tc.tile_pool(name="psg", bufs=1, space="PSUM") as psg:
            gsum = psg.tile([GPC, 2 * KC], F32, tag="gsum")
            nc.tensor.matmul(gsum, lhsT=ind, rhs=statsm_r, start=True, stop=True)
            # per-group mean / E[x^2] (divide the 16-channel sums)
            gsb = main.tile([GPC, 2, KC], F32, tag="gsb")
            nc.vector.tensor_scalar_mul(gsb[:, :, :], gsum[:, :], 1.0 / GS)
            tmp = main.tile([GPC, KC], F32, tag="gtmp")
            nc.vector.tensor_tensor(tmp, gsb[:, 0, :], gsb[:, 0, :], OP.mult)
            nc.vector.tensor_tensor(gsb[:, 1, :], gsb[:, 1, :], tmp, OP.subtract)
            nc.scalar.activation(
                out=gsb[:, 1, :], in_=gsb[:, 1, :], func=AF.Sqrt, bias=eps_t[:, :]
            )
            nc.vector.reciprocal(out=gsb[:, 1, :], in_=gsb[:, 1, :])
            bc_in = main.tile([GPC, 2 * KC], F32R, tag="bc_in")
            nc.vector.tensor_copy(out=bc_in, in_=gsb)
            bb = psg.tile([P, 2 * KC], F32, tag="bb")
            nc.tensor.matmul(bb, lhsT=indT, rhs=bc_in, start=True, stop=True)
            # a = gn_w * rstd ; b2 = gn_b - mu * a
            nc.vector.tensor_tensor(a_t, gw_t, bb[:, KC : 2 * KC], OP.mult)
            nc.vector.tensor_tensor(b2_t, bb[:, 0:KC], a_t, OP.mult)
            nc.vector.tensor_tensor(b2_t, gb_t, b2_t, OP.subtract)

        # hn = x * a + b2, rounded to f32r in place
        hn = x_t
        for kc in range(KC):
            nc.vector.tensor_scalar(
                hn[:, kc, :],
                xf[:, kc, :],
                a_t[:, kc : kc + 1],
                b2_t[:, kc : kc + 1],
                OP.mult,
                OP.add,
            )

        # ---------------- V^T, K, Q projections ----------------
        vt_t = main.tile([P, JT, C], BF16, tag="vt")
        k_t = main.tile([P, KC, N], F32R, tag="kt")
        q_t = main.tile([P, KC, ISL], F32R, tag="qt")

        wv_t = wpool.tile([P, KC, C], F32R, tag="w")
        nc.sync.dma_start(out=wv_t, in_=wvd[:, :].rearrange("(kc p) c -> p kc c", p=P))
        wk_t = wpool.tile([P, KC, C], F32R, tag="w")
        nc.sync.dma_start(out=wk_t, in_=wkd[:, :].rearrange("(kc p) c -> p kc c", p=P))

        with tc.tile_pool(name="psq", bufs=8, space="PSUM") as psq:
            for jt in range(JT):
                ps = psq.tile([P, C], F32, tag="ps")
                for kc in range(KC):
                    nc.tensor.matmul(
                        ps,
                        lhsT=hn[:, kc, jt * P : (jt + 1) * P],
                        rhs=wv_t[:, kc, :],
                        start=(kc == 0),
                        stop=(kc == KC - 1),
                    )
                nc.vector.tensor_tensor(vt_t[:, jt, :], ps, bv_b, OP.add)

            for co in range(KC):
                for jn in range(JN):
                    ps = psq.tile([P, 512], F32, tag="ps")
                    for kc in range(KC):
                        nc.tensor.matmul(
                            ps,
                            lhsT=wk_t[:, kc, co * P : (co + 1) * P],
                            rhs=hn[:, kc, jn * 512 : (jn + 1) * 512],
                            start=(kc == 0),
                            stop=(kc == KC - 1),
                        )
                    nc.scalar.activation(
                        out=k_t[:, co, jn * 512 : (jn + 1) * 512],
                        in_=ps,
                        func=AF.Identity,
                        bias=bk_t[:, co : co + 1],
                    )

            wq_t = main.tile([P, KC, C], F32R, tag="osb", name="wq_t")
            nc.sync.dma_start(
                out=wq_t, in_=wqd[:, :].rearrange("(kc p) c -> p kc c", p=P)
            )
            for co in range(KC):
                for ic in range(IC):
                    ps = psq.tile([P, 512], F32, tag="ps")
                    for kc in range(KC):
                        nc.tensor.matmul(
                            ps,
                            lhsT=wq_t[:, kc, co * P : (co + 1) * P],
                            rhs=hn[:, kc, ic * 512 : (ic + 1) * 512],
                            start=(kc == 0),
                            stop=(kc == KC - 1),
                        )
                    nc.scalar.activation(
                        out=q_t[:, co, ic * 512 : (ic + 1) * 512],
                        in_=ps,
                        func=AF.Identity,
                        bias=bq_t[:, co : co + 1],
                    )

        # ---------------- attention ----------------
        ones_t = main.tile([P, 1], BF16, tag="ones")
        nc.vector.memset(ones_t, 1.0)
        ones_col = main.tile([1, P], F32R, tag="ones_col")
        nc.sync.dma_start(out=ones_col, in_=onesd[:, :])
        # scratch reuses x/hn's 64KB slot: xres | P-out, 1024 each
        scratch = main.tile([P, KC, 2 * ISL], F32, tag="bigA")
        scr_r = scratch.bitcast(F32R)
        o_sb = main.tile([P, KC, ISL], F32R, tag="osb")
        linv_b = main.tile([P, 512], BF16, tag="linvb")

        for kc in range(KC):
            nc.sync.dma_start(out=scr_r[:, kc, 0:ISL], in_=xre[:, kc, 0:ISL])

        with tc.tile_pool(name="psa", bufs=1, space="PSUM") as psa:
            for ic in range(IC):
                l_ps = psa.tile([1, 512], F32, tag="l")
                o_ps = [
                    psa.tile([P, 512], F32, tag=f"o{co}", name=f"o_ps{co}")
                    for co in range(KC)
                ]
                for jt in range(JT):
                    st = psa.tile([P, 512], F32, tag="st", bufs=2)
                    for kc in range(KC):
                        nc.tensor.matmul(
                            st,
                            lhsT=k_t[:, kc, jt * P : (jt + 1) * P],
                            rhs=q_t[:, kc, ic * 512 : (ic + 1) * 512],
                            start=(kc == 0),
                            stop=(kc == KC - 1),
                        )
                    et = etp.tile([P, 512], BF16, tag="et")
                    nc.scalar.activation(out=et, in_=st, func=AF.Exp, scale=SCALE)
                    nc.tensor.matmul(
                        l_ps,
                        lhsT=ones_t,
                        rhs=et,
                        start=(jt == 0),
                        stop=(jt == JT - 1),
                    )
                    for co in range(KC):
                        nc.tensor.matmul(
                            o_ps[co],
                            lhsT=vt_t[:, jt, co * P : (co + 1) * P],
                            rhs=et,
                            start=(jt == 0),
                            stop=(jt == JT - 1),
                        )
                # scratch the [1,512] reciprocal into o_sb's slot for this
                # i-chunk (consumed by the broadcast matmul before O-norm
                # overwrites it)
                linv_1 = o_sb[0:1, 0, ic * 512 : (ic + 1) * 512]
                with nc.allow_low_precision(
                    reason="f32r rounding of softmax 1/l is intentional"
                ):
                    nc.vector.reciprocal(out=linv_1, in_=l_ps)
                lb_ps = psa.tile([P, 512], F32, tag="lb", name="lb_ps")
                nc.tensor.matmul(lb_ps, lhsT=ones_col, rhs=linv_1, start=True, stop=True)
                nc.scalar.activation(out=linv_b, in_=lb_ps, func=AF.Copy)
                for co in range(KC):
                    nc.vector.tensor_tensor(
                        o_sb[:, co, ic * 512 : (ic + 1) * 512],
                        o_ps[co],
                        linv_b,
                        OP.mult,
                    )

        # ---------------- output projection + residual ----------------
        wp_t = main.tile([P, KC, C], F32R, tag="qt", name="wp_t")
        nc.sync.dma_start(
            out=wp_t, in_=wpd[:, :].rearrange("(kc p) c -> p kc c", p=P)
        )
        with tc.tile_pool(name="psp", bufs=2, space="PSUM") as psp:
            for co in range(KC):
                for ic in range(IC):
                    ps = psp.tile([P, 512], F32, tag="ps")
                    for kc in range(KC):
                        nc.tensor.matmul(
                            ps,
                            lhsT=wp_t[:, kc, co * P : (co + 1) * P],
                            rhs=o_sb[:, kc, ic * 512 : (ic + 1) * 512],
                            start=(kc == 0),
                            stop=(kc == KC - 1),
                        )
                    dst = scratch[:, co, ISL + ic * 512 : ISL + (ic + 1) * 512]
                    nc.scalar.activation(
                        out=dst, in_=ps, func=AF.Identity, bias=bp_t[:, co : co + 1]
                    )
                    nc.vector.tensor_tensor(
                        dst, dst, scratch[:, co, ic * 512 : (ic + 1) * 512], OP.add
                    )
        nc.sync.dma_start(
            out=od[:, :].rearrange("(kc p) i -> p kc i", p=P),
            in_=scratch[:, :, ISL : 2 * ISL],
        )


_NC_CACHE = {}


def _get_nc():
    if "nc" not in _NC_CACHE:
        nc = bacc.Bacc(trn_type="TRN2", target_bir_lowering=False, num_devices=NCORES)
        with tile.TileContext(nc) as tc:
            _emit(nc, tc)
        nc.compile()
        _NC_CACHE["nc"] = nc
    return _NC_CACHE["nc"]


def kernel(x, gn_w, gn_b, wq, bq, wk, bk, wv, bv, wp, bp, _trace=False):
    x = np.asarray(x, dtype=np.float32)
    to_pkc = lambda v: np.ascontiguousarray(
        np.asarray(v, dtype=np.float32).reshape(KC, P).T
    )
    shared = {
        "wqT": np.ascontiguousarray(np.asarray(wq, np.float32).T),
        "wkT": np.ascontiguousarray(np.asarray(wk, np.float32).T),
        "wvT": np.ascontiguousarray(np.asarray(wv, np.float32).T),
        "wpT": np.ascontiguousarray(np.asarray(wp, np.float32).T),
        "bq": to_pkc(bq),
        "bk": to_pkc(bk),
        "bp": to_pkc(bp),
        "bv_row": np.ascontiguousarray(np.asarray(bv, np.float32).reshape(1, C)),
        "gnw": to_pkc(gn_w),
        "gnb": to_pkc(gn_b),
        "ind": np.ascontiguousarray(
            np.kron(np.eye(P // GS), np.ones((GS, 1))).astype(np.float32)
        ),
        "indT": np.ascontiguousarray(
            np.kron(np.eye(P // GS), np.ones((1, GS))).astype(np.float32)
        ),
        "ones_col": np.ones((1, P), np.float32),
    }
    in_maps = []
    for b in range(B):
        xb = np.ascontiguousarray(x[b].reshape(C, N))
        for s in range(SLICES):
            off = s * ISL
            xroll = xb if off == 0 else np.ascontiguousarray(np.roll(xb, -off, axis=1))
            in_maps.append({"x": xroll, **shared})

    nc = _get_nc()
    res = run_bass_kernel_spmd(
        nc, in_maps, core_ids=list(range(NCORES)), trace=_trace
    )
    out = np.empty((B, C, N), np.float32)
    for idx in range(NCORES):
        b, s = divmod(idx, SLICES)
        out[b][:, s * ISL : (s + 1) * ISL] = res.results[idx]["out"]
    out = out.reshape(B, C, 16, 16, 16)
    if _trace:
        return out, res
    return out
